# revision 32
# baseline (speedup 1.0000x reference)
"""Trainium2 Bass kernel for EquivariantTPConv (gnn_message_passing).

Computation per edge e:
  sh  = SH_l012(edge_vec[e])                                  # [9]
  w   = (silu(edge_scalars[e] @ W1 + b1) @ W2 + b2)           # [3*64*16]
  x   = h_src[src_idx[e]]                                     # [64]
  feat[l,v] = sum_u x[u] * w[l,u,v] / 8                       # [3,16]
  msg = concat_l (feat[:,l,:,None] * sh_l[None,:])            # [144]
  out[d] = mean over {e: dst_idx[e]==d} msg[e]                # [n_dst,144]

Strategy (8 NeuronCores, edge/data parallel):
  - Host shards edges into 8 contiguous chunks, sorts each shard by dst,
    gathers h_src rows per shard (sharding/data movement only), and builds
    per-tile one-hot "slot" matrices (slot = distinct dst within a 128-edge
    tile) plus 1/count scales. All per-core arrays are stored partition-major
    so each lands on the device in one big DMA.
  - Device (per core): SH for all edges upfront in wide [128, NT, k] ops
    (Vector+Scalar engines); MLP on TensorE in bf16 (h1^T = W1^T @ es^T so
    the silu output is directly the lhsT of the big matmul); w = h2 @ W2perm
    on TensorE (fp32 PSUM); ScalarE casts w to bf16 SBUF; VectorE multiplies
    by the gathered x (2x bf16 mode); GpSimd adds the two u-halves; VectorE
    reduces the remaining 32; one-hot matmul on TensorE does the within-tile
    segment-sum; 1/count is applied during the PSUM->SBUF eviction.
  - Host unshard: segment-sum of the (sorted-label) slot rows across tiles
    and cores via np.add.reduceat == the cross-core all-reduce step.

W2 is column-permuted (l,u,v)->(l,v,u) and pre-scaled by 1/sqrt(64) on the
host so u is contiguous for the contraction; b2 likewise reshaped to [64,48]
and handled exactly via one extra tiny matmul x @ b2r per tile.
"""

import sys

for _p in ("/opt/trn_rl_repo", "/root/.axon_site/_ro/trn_rl_repo"):
    if _p not in sys.path:
        sys.path.append(_p)

import numpy as np

MUL_SRC = 64
MUL_DST = 16
N_PATHS = 3
SQ3 = 3.0 ** 0.5
SQ5 = 5.0 ** 0.5
SQ15 = 15.0 ** 0.5

N_CORES = 8
E_TOT = 50000
N_SRC = 10000
N_DST = 10000
ESD = 32
HID = 128
WCOLS = N_PATHS * MUL_DST * MUL_SRC  # 3072 (perm layout (l,v,u))
NLV = N_PATHS * MUL_DST  # 48

T = 128  # edges per tile
BLK = 512  # edges per full MM1 block (last block is a 128-edge tail)
EC = E_TOT // N_CORES  # 6250 edges per core
NT = (EC + T - 1) // T  # 49 tiles
EP = NT * T  # 6272 padded edges per core
NB = (EP + BLK - 1) // BLK  # 13 blocks, last one covers a single tile

_PROG = None  # cached compiled program


def _build_program():
    from contextlib import ExitStack

    import concourse.tile as tile
    from concourse import bacc, mybir

    f32 = mybir.dt.float32
    bf16 = mybir.dt.bfloat16
    AF = mybir.ActivationFunctionType
    OP = mybir.AluOpType
    AX = mybir.AxisListType

    nc = bacc.Bacc(
        "TRN2",
        target_bir_lowering=False,
        debug=False,
        enable_asserts=False,
        num_devices=N_CORES,
    )

    # DRAM inputs; all big per-core tensors are partition-major on the host.
    esT_d = nc.dram_tensor("esT", [ESD, EP], bf16, kind="ExternalInput")
    x_d = nc.dram_tensor("xg", [T, NT, MUL_SRC], bf16, kind="ExternalInput")
    xT_d = nc.dram_tensor("xgT", [MUL_SRC, EP], bf16, kind="ExternalInput")
    ev_d = nc.dram_tensor("ev", [T, NT, 3], f32, kind="ExternalInput")
    oh_d = nc.dram_tensor("oh", [T, NT, T], bf16, kind="ExternalInput")
    isl_d = nc.dram_tensor("isl", [T, NT], f32, kind="ExternalInput")
    W1_d = nc.dram_tensor("W1", [ESD, HID], bf16, kind="ExternalInput")
    b1_d = nc.dram_tensor("b1", [HID, 1], f32, kind="ExternalInput")
    W2_d = nc.dram_tensor("W2p", [HID, WCOLS], bf16, kind="ExternalInput")
    b2_d = nc.dram_tensor("b2r", [MUL_SRC, NLV], bf16, kind="ExternalInput")
    shc_d = nc.dram_tensor("shc", [T, 8], f32, kind="ExternalInput")
    out_d = nc.dram_tensor("outp", [T, NT, 144], f32, kind="ExternalOutput")

    with ExitStack() as ctx:
        tc = ctx.enter_context(tile.TileContext(nc))

        const = ctx.enter_context(tc.tile_pool(name="const", bufs=1))
        shp = ctx.enter_context(tc.tile_pool(name="shp", bufs=1))
        h2pool = ctx.enter_context(tc.tile_pool(name="h2pool", bufs=3))
        prodp = ctx.enter_context(tc.tile_pool(name="prodp", bufs=6))
        msgp = ctx.enter_context(tc.tile_pool(name="msgp", bufs=5))
        ph1 = ctx.enter_context(tc.tile_pool(name="ph1", bufs=1, space="PSUM"))
        pwp = ctx.enter_context(tc.tile_pool(name="pwp", bufs=2, space="PSUM"))
        psp = ctx.enter_context(tc.tile_pool(name="psp", bufs=2, space="PSUM"))
        pxb = ctx.enter_context(tc.tile_pool(name="pxb", bufs=1, space="PSUM"))

        # ---- resident loads; SP carries the edge streams (it has no compute),
        # GpSimd's queue carries the weights, ordered by first use ----
        W1s = const.tile([ESD, HID], bf16)
        nc.gpsimd.dma_start(W1s[:], W1_d.ap())
        b1s = const.tile([HID, 1], f32)
        nc.gpsimd.dma_start(b1s[:], b1_d.ap())
        W2s = const.tile([HID, WCOLS], bf16)
        nc.gpsimd.dma_start(W2s[:, 0:1024], W2_d.ap()[:, 0:1024])
        nc.gpsimd.dma_start(W2s[:, 1024:], W2_d.ap()[:, 1024:])
        b2s = const.tile([MUL_SRC, NLV], bf16)
        nc.gpsimd.dma_start(b2s[:], b2_d.ap())
        shcs = const.tile([T, 8], f32)
        nc.gpsimd.dma_start(shcs[:], shc_d.ap())

        es_all = const.tile([ESD, EP], bf16)
        nc.sync.dma_start(es_all[:, 0:BLK], esT_d.ap()[:, 0:BLK])
        x_all = const.tile([T, NT, MUL_SRC], bf16)
        nc.sync.dma_start(x_all[:, 0:4, :], x_d.ap()[:, 0:4, :])
        ev_all = const.tile([T, NT, 3], f32)
        nc.sync.dma_start(ev_all[:], ev_d.ap())
        isl_all = const.tile([T, NT], f32)
        nc.sync.dma_start(isl_all[:], isl_d.ap())
        oh_all = const.tile([T, NT, T], bf16)
        nc.sync.dma_start(oh_all[:, 0:4, :], oh_d.ap()[:, 0:4, :])
        xT_all = const.tile([MUL_SRC, EP], bf16)
        nc.sync.dma_start(xT_all[:, 0 : 4 * T], xT_d.ap()[:, 0 : 4 * T])
        nc.sync.dma_start(es_all[:, BLK:], esT_d.ap()[:, BLK:])
        nc.sync.dma_start(x_all[:, 4:, :], x_d.ap()[:, 4:, :])
        nc.sync.dma_start(oh_all[:, 4:, :], oh_d.ap()[:, 4:, :])
        nc.sync.dma_start(xT_all[:, 4 * T :], xT_d.ap()[:, 4 * T :])
        negone = const.tile([T, 1], f32)
        nc.vector.memset(negone[:], -1.0)

        ob_all = const.tile([T, NT, 144], f32)

        # ---- SH prologue: all edges at once, [128, NT, k] layouts ----
        sq_all = shp.tile([T, NT, 3], f32)
        nc.vector.tensor_tensor(sq_all[:], ev_all[:], ev_all[:], op=OP.mult)
        r2_all = shp.tile([T, NT], f32)
        nc.vector.tensor_reduce(r2_all[:], sq_all[:], axis=AX.X, op=OP.add)
        rn_all = shp.tile([T, NT], f32)
        nc.scalar.activation(rn_all[:], r2_all[:], AF.Sqrt)
        inv_all = shp.tile([T, NT], f32)
        nc.vector.reciprocal(inv_all[:], rn_all[:])
        inv2_all = shp.tile([T, NT], f32)
        nc.vector.tensor_tensor(inv2_all[:], inv_all[:], inv_all[:], op=OP.mult)

        def bc(ap_, shape):
            return ap_.to_broadcast(shape)

        sh_all = shp.tile([T, NT, 9], f32)
        i1 = inv_all[:].rearrange("p (t o) -> p t o", o=1)
        i2 = inv2_all[:].rearrange("p (t o) -> p t o", o=1)
        nc.vector.tensor_tensor(
            sh_all[:, :, 1:4], ev_all[:], bc(i1, [T, NT, 3]), op=OP.mult
        )
        pq_all = shp.tile([T, NT, 2], f32)
        nc.vector.tensor_tensor(
            pq_all[:], ev_all[:, :, 0:2], ev_all[:, :, 1:3], op=OP.mult
        )
        nc.vector.tensor_tensor(
            sh_all[:, :, 4:6], pq_all[:], bc(i2, [T, NT, 2]), op=OP.mult
        )
        t6_all = shp.tile([T, NT], f32)
        nc.vector.tensor_tensor(
            t6_all[:].rearrange("p (t o) -> p t o", o=1),
            sq_all[:, :, 2:3],
            i2,
            op=OP.mult,
        )
        nc.scalar.activation(
            sh_all[:, :, 6], t6_all[:], AF.Identity, bias=negone[:, 0:1], scale=3.0
        )
        xz_all = shp.tile([T, NT, 1], f32)
        nc.vector.tensor_tensor(
            xz_all[:], ev_all[:, :, 0:1], ev_all[:, :, 2:3], op=OP.mult
        )
        nc.vector.tensor_tensor(sh_all[:, :, 7:8], xz_all[:], i2, op=OP.mult)
        d2_all = shp.tile([T, NT, 1], f32)
        nc.vector.tensor_tensor(
            d2_all[:], sq_all[:, :, 0:1], sq_all[:, :, 1:2], op=OP.subtract
        )
        nc.vector.tensor_tensor(sh_all[:, :, 8:9], d2_all[:], i2, op=OP.mult)
        shc3 = shcs[:].rearrange("p (o c) -> p o c", o=1)
        nc.vector.tensor_tensor(
            sh_all[:, :, 1:9], sh_all[:, :, 1:9], bc(shc3, [T, NT, 8]), op=OP.mult
        )

        # ---- main pipeline: head(t) emits matmuls/cast/mul/half; the tail
        # (reduce/msg/scatter/evict) is deferred one tile so each engine's
        # in-order stream never waits on the producing engine's latest op.
        xb_by_block = {}

        def tail(st):
            t, halves, tt = st
            xb = xb_by_block[t // 4]
            featc = msgp.tile([T, NLV], f32, tag="featc", name=f"featc{t}")
            for l in range(3):
                nc.vector.tensor_reduce(
                    featc[:, l * MUL_DST : (l + 1) * MUL_DST],
                    halves[l][:],
                    axis=AX.X,
                    op=OP.add,
                )
            # + x @ b2r
            nc.vector.tensor_tensor(
                featc[:], featc[:], xb[:, tt * NLV : (tt + 1) * NLV], op=OP.add
            )
            msg = msgp.tile([T, 144], bf16, tag="msg", name=f"msg{t}")
            nc.vector.tensor_copy(msg[:, 0:16], featc[:, 0:16])
            nc.gpsimd.tensor_tensor(
                msg[:, 16:64].rearrange("p (v m) -> p v m", m=3),
                featc[:, 16:32]
                .rearrange("p (v o) -> p v o", o=1)
                .to_broadcast([T, 16, 3]),
                sh_all[:, t, 1:4]
                .rearrange("p (o m) -> p o m", o=1)
                .to_broadcast([T, 16, 3]),
                op=OP.mult,
            )
            nc.gpsimd.tensor_tensor(
                msg[:, 64:144].rearrange("p (v m) -> p v m", m=5),
                featc[:, 32:48]
                .rearrange("p (v o) -> p v o", o=1)
                .to_broadcast([T, 16, 5]),
                sh_all[:, t, 4:9]
                .rearrange("p (o m) -> p o m", o=1)
                .to_broadcast([T, 16, 5]),
                op=OP.mult,
            )
            ps = psp.tile([T, 144], f32, tag="ps", name=f"ps{t}")
            nc.tensor.matmul(ps[:], oh_all[:, t, :], msg[:], start=True, stop=True)
            nc.scalar.activation(
                ob_all[:, t, :], ps[:], AF.Copy, scale=isl_all[:, t : t + 1]
            )

        pending = []
        for t in range(NT):
            b, tt = divmod(t, 4)
            nbt = min(4, NT - b * 4)  # tiles in this block
            bw = nbt * T  # block width in edges
            if tt == 0:
                h1 = ph1.tile([HID, bw], f32, tag="h1", name=f"h1_{b}")
                nc.tensor.matmul(
                    h1[:],
                    W1s[:],
                    es_all[:, b * BLK : b * BLK + bw],
                    start=True,
                    stop=True,
                )
                h2 = h2pool.tile([HID, bw], bf16, tag="h2", name=f"h2_{b}")
                nc.scalar.activation(h2[:], h1[:], AF.Silu, bias=b1s[:, 0:1])
            if tt == min(1, nbt - 1):
                xbp = pxb.tile([T, 4 * NLV], f32, tag="xbp", name=f"xbp{b}")
                for q in range(nbt):
                    nc.tensor.matmul(
                        xbp[:, q * NLV : (q + 1) * NLV],
                        xT_all[:, (b * 4 + q) * T : (b * 4 + q + 1) * T],
                        b2s[:],
                        start=True,
                        stop=True,
                    )
                xb = msgp.tile([T, 4 * NLV], f32, tag="xb", name=f"xb{b}")
                nc.vector.tensor_copy(xb[:], xbp[:])
                xb_by_block[b] = xb

            lhs = h2[:, tt * T : (tt + 1) * T]
            xv = x_all[:, t : t + 1, :].to_broadcast([T, MUL_DST, MUL_SRC])
            prods = []
            halves = []
            for l in range(3):
                pw = pwp.tile([T, 1024], f32, tag="pw", name=f"pw{t}_{l}")
                for h in range(2):
                    nc.tensor.matmul(
                        pw[:, h * 512 : (h + 1) * 512],
                        lhs,
                        W2s[:, l * 1024 + h * 512 : l * 1024 + (h + 1) * 512],
                        start=True,
                        stop=True,
                    )
                wb = prodp.tile([T, 1024], bf16, tag="wb", name=f"wb{t}_{l}")
                nc.scalar.activation(wb[:], pw[:], AF.Copy)
                prod = prodp.tile([T, 1024], bf16, tag="prod", name=f"prod{t}_{l}")
                wb3 = wb.rearrange("p (v u) -> p v u", u=MUL_SRC)
                pr3 = prod.rearrange("p (v u) -> p v u", u=MUL_SRC)
                nc.vector.tensor_tensor(pr3, wb3, xv, op=OP.mult)
                prods.append(pr3)
            for l in range(3):
                half = prodp.tile(
                    [T, MUL_DST, 32], f32, tag="half", bufs=12, name=f"half{t}_{l}"
                )
                nc.gpsimd.tensor_tensor(
                    half[:], prods[l][:, :, 0:32], prods[l][:, :, 32:64], op=OP.add
                )
                halves.append(half)

            pending.append((t, halves, tt))
            if len(pending) > 2:
                tail(pending.pop(0))
        for st in pending:
            tail(st)

        # output: chunked DMAs, small final chunk so the tail drains fast
        bounds = [0, 12, 24, 36, 44, 48, NT]
        for c in range(len(bounds) - 1):
            nc.sync.dma_start(
                out_d.ap()[:, bounds[c] : bounds[c + 1], :],
                ob_all[:, bounds[c] : bounds[c + 1], :],
            )

    nc.compile()
    return nc


def _get_program():
    global _PROG
    if _PROG is None:
        _PROG = _build_program()
    return _PROG


def _prep_core(c, h_src, edge_vec, edge_scalars, src_idx, dst_idx, inv_cnt):
    """Shard + sort + gather + one-hot build for one core (partition-major)."""
    import ml_dtypes

    bf = ml_dtypes.bfloat16
    lo, hi = c * EC, (c + 1) * EC
    d = dst_idx[lo:hi]
    order = np.argsort(d, kind="stable")
    d_s = d[order]
    s_s = src_idx[lo:hi][order]

    esT = np.zeros((ESD, EP), np.float32)
    esT[:, :EC] = edge_scalars[lo:hi][order].T
    x = np.zeros((EP, MUL_SRC), np.float32)
    x[:EC] = h_src[s_s]
    ev = np.zeros((EP, 3), np.float32)
    ev[:EC] = edge_vec[lo:hi][order]
    ev[EC:, 0] = 1.0

    d_pad = np.full(EP, N_DST, np.int64)
    d_pad[:EC] = d_s

    oh = np.zeros((EP, T), np.float32)
    isl = np.ones((EP,), np.float32)
    labels = np.full(NT * T, N_DST, np.int64)
    dt2 = d_pad.reshape(NT, T)
    for t in range(NT):
        uniq, inv = np.unique(dt2[t], return_inverse=True)
        oh[t * T : (t + 1) * T, :][np.arange(T), inv] = 1.0
        labels[t * T : t * T + len(uniq)] = uniq
        real = uniq[uniq < N_DST]
        isl[t * T : t * T + len(real)] = inv_cnt[real]

    # partition-major device layouts: [p, t, ...] = row t*T + p
    def pmaj(a):
        return np.ascontiguousarray(a.reshape(NT, T, -1).transpose(1, 0, 2))

    return (
        {
            "esT": esT.astype(bf),
            "xg": pmaj(x).astype(bf),
            "xgT": np.ascontiguousarray(x.T).astype(bf),
            "ev": pmaj(ev),
            "oh": pmaj(oh).astype(bf),
            "isl": np.ascontiguousarray(isl.reshape(NT, T).T),
        },
        labels,
    )


def kernel(**inputs):
    import ml_dtypes

    from concourse import bass_utils

    bf = ml_dtypes.bfloat16

    h_src = np.asarray(inputs["h_src"], np.float32)
    edge_vec = np.asarray(inputs["edge_vec"], np.float32)
    edge_scalars = np.asarray(inputs["edge_scalars"], np.float32)
    W1 = np.asarray(inputs["W1"], np.float32)
    b1 = np.asarray(inputs["b1"], np.float32)
    W2 = np.asarray(inputs["W2"], np.float32)
    b2 = np.asarray(inputs["b2"], np.float32)
    src_idx = np.asarray(inputs["src_idx"]).astype(np.int64)
    dst_idx = np.asarray(inputs["dst_idx"]).astype(np.int64)
    n_dst = int(inputs["n_dst"])
    assert n_dst == N_DST

    nc = _get_program()

    cnt = np.bincount(dst_idx, minlength=N_DST)
    inv_cnt = (1.0 / np.maximum(cnt, 1)).astype(np.float32)

    # weights in (l,v,u) column order, pre-scaled by 1/sqrt(64)
    scale = 1.0 / np.sqrt(MUL_SRC)
    W2p = (
        W2.reshape(HID, N_PATHS, MUL_SRC, MUL_DST).transpose(0, 1, 3, 2) * scale
    ).reshape(HID, WCOLS)
    b2r = (b2.reshape(N_PATHS, MUL_SRC, MUL_DST).transpose(1, 0, 2) * scale).reshape(
        MUL_SRC, NLV
    )

    shc = np.broadcast_to(
        np.array(
            [SQ3, SQ3, SQ3, SQ15, SQ15, 0.5 * SQ5, SQ15, 0.5 * SQ15], np.float32
        ),
        (T, 8),
    ).copy()

    shared = {
        "W1": np.ascontiguousarray(W1).astype(bf),
        "b1": b1.reshape(HID, 1).astype(np.float32),
        "W2p": W2p.astype(bf),
        "b2r": b2r.astype(bf),
        "shc": shc,
    }

    in_maps = []
    labels_all = []
    for c in range(N_CORES):
        m, labels = _prep_core(
            c, h_src, edge_vec, edge_scalars, src_idx, dst_idx, inv_cnt
        )
        m.update(shared)
        in_maps.append(m)
        labels_all.append(labels)

    import time

    t0 = time.perf_counter()
    res = bass_utils.run_bass_kernel_spmd(nc, in_maps, core_ids=list(range(N_CORES)))
    t1 = time.perf_counter()
    kernel.last_device_wall_s = t1 - t0

    # outp is [T, NT, 144] partition-major; row (t, p) lives at [p, t, :]
    rows = np.concatenate(
        [
            res.results[c]["outp"].transpose(1, 0, 2).reshape(NT * T, 144)
            for c in range(N_CORES)
        ],
        axis=0,
    )
    labels = np.concatenate(labels_all)

    order = np.argsort(labels, kind="stable")
    lab_s = labels[order]
    rows_s = rows[order]
    starts = np.concatenate(([0], np.flatnonzero(np.diff(lab_s)) + 1))
    sums = np.add.reduceat(rows_s, starts, axis=0)
    out = np.zeros((N_DST + 1, 144), np.float32)
    out[lab_s[starts]] = sums
    return out[:N_DST]


# revision 33
# speedup vs baseline: 1.0015x; 1.0015x over previous
"""Trainium2 Bass kernel for EquivariantTPConv (gnn_message_passing).

Computation per edge e:
  sh  = SH_l012(edge_vec[e])                                  # [9]
  w   = (silu(edge_scalars[e] @ W1 + b1) @ W2 + b2)           # [3*64*16]
  x   = h_src[src_idx[e]]                                     # [64]
  feat[l,v] = sum_u x[u] * w[l,u,v] / 8                       # [3,16]
  msg = concat_l (feat[:,l,:,None] * sh_l[None,:])            # [144]
  out[d] = mean over {e: dst_idx[e]==d} msg[e]                # [n_dst,144]

Strategy (8 NeuronCores, edge/data parallel):
  - Host shards edges into 8 contiguous chunks, sorts each shard by dst,
    gathers h_src rows per shard (sharding/data movement only), and builds
    per-tile one-hot "slot" matrices (slot = distinct dst within a 128-edge
    tile) plus 1/count scales. All per-core arrays are stored partition-major
    so each lands on the device in one big DMA.
  - Device (per core): SH for all edges upfront in wide [128, NT, k] ops
    (Vector+Scalar engines); MLP on TensorE in bf16 (h1^T = W1^T @ es^T so
    the silu output is directly the lhsT of the big matmul); w = h2 @ W2perm
    on TensorE (fp32 PSUM); ScalarE casts w to bf16 SBUF; VectorE multiplies
    by the gathered x (2x bf16 mode); GpSimd adds the two u-halves; VectorE
    reduces the remaining 32; one-hot matmul on TensorE does the within-tile
    segment-sum; 1/count is applied during the PSUM->SBUF eviction.
  - Host unshard: segment-sum of the (sorted-label) slot rows across tiles
    and cores via np.add.reduceat == the cross-core all-reduce step.

W2 is column-permuted (l,u,v)->(l,v,u) and pre-scaled by 1/sqrt(64) on the
host so u is contiguous for the contraction; b2 likewise reshaped to [64,48]
and handled exactly via one extra tiny matmul x @ b2r per tile.
"""

import sys

for _p in ("/opt/trn_rl_repo", "/root/.axon_site/_ro/trn_rl_repo"):
    if _p not in sys.path:
        sys.path.append(_p)

import numpy as np

MUL_SRC = 64
MUL_DST = 16
N_PATHS = 3
SQ3 = 3.0 ** 0.5
SQ5 = 5.0 ** 0.5
SQ15 = 15.0 ** 0.5

N_CORES = 8
E_TOT = 50000
N_SRC = 10000
N_DST = 10000
ESD = 32
HID = 128
WCOLS = N_PATHS * MUL_DST * MUL_SRC  # 3072 (perm layout (l,v,u))
NLV = N_PATHS * MUL_DST  # 48

T = 128  # edges per tile
BLK = 512  # edges per full MM1 block (last block is a 128-edge tail)
EC = E_TOT // N_CORES  # 6250 edges per core
NT = (EC + T - 1) // T  # 49 tiles
EP = NT * T  # 6272 padded edges per core
NB = (EP + BLK - 1) // BLK  # 13 blocks, last one covers a single tile

_PROG = None  # cached compiled program


def _build_program():
    from contextlib import ExitStack

    import concourse.tile as tile
    from concourse import bacc, mybir

    f32 = mybir.dt.float32
    bf16 = mybir.dt.bfloat16
    AF = mybir.ActivationFunctionType
    OP = mybir.AluOpType
    AX = mybir.AxisListType

    nc = bacc.Bacc(
        "TRN2",
        target_bir_lowering=False,
        debug=False,
        enable_asserts=False,
        num_devices=N_CORES,
    )

    # DRAM inputs; all big per-core tensors are partition-major on the host.
    esT_d = nc.dram_tensor("esT", [ESD, EP], bf16, kind="ExternalInput")
    x_d = nc.dram_tensor("xg", [T, NT, MUL_SRC], bf16, kind="ExternalInput")
    xT_d = nc.dram_tensor("xgT", [MUL_SRC, EP], bf16, kind="ExternalInput")
    ev_d = nc.dram_tensor("ev", [T, NT, 3], f32, kind="ExternalInput")
    oh_d = nc.dram_tensor("oh", [T, NT, T], bf16, kind="ExternalInput")
    isl_d = nc.dram_tensor("isl", [T, NT], f32, kind="ExternalInput")
    W1_d = nc.dram_tensor("W1", [ESD, HID], bf16, kind="ExternalInput")
    b1_d = nc.dram_tensor("b1", [HID, 1], f32, kind="ExternalInput")
    W2_d = nc.dram_tensor("W2p", [HID, WCOLS], bf16, kind="ExternalInput")
    b2_d = nc.dram_tensor("b2r", [MUL_SRC, NLV], bf16, kind="ExternalInput")
    shc_d = nc.dram_tensor("shc", [T, 8], f32, kind="ExternalInput")
    out_d = nc.dram_tensor("outp", [T, NT, 144], f32, kind="ExternalOutput")

    with ExitStack() as ctx:
        tc = ctx.enter_context(tile.TileContext(nc))

        const = ctx.enter_context(tc.tile_pool(name="const", bufs=1))
        shp = ctx.enter_context(tc.tile_pool(name="shp", bufs=1))
        h2pool = ctx.enter_context(tc.tile_pool(name="h2pool", bufs=4))
        prodp = ctx.enter_context(tc.tile_pool(name="prodp", bufs=8))
        msgp = ctx.enter_context(tc.tile_pool(name="msgp", bufs=6))
        ph1 = ctx.enter_context(tc.tile_pool(name="ph1", bufs=1, space="PSUM"))
        pwp = ctx.enter_context(tc.tile_pool(name="pwp", bufs=2, space="PSUM"))
        psp = ctx.enter_context(tc.tile_pool(name="psp", bufs=2, space="PSUM"))
        pxb = ctx.enter_context(tc.tile_pool(name="pxb", bufs=1, space="PSUM"))

        # ---- resident loads; SP carries the edge streams (it has no compute),
        # GpSimd's queue carries the weights, ordered by first use ----
        W1s = const.tile([ESD, HID], bf16)
        nc.gpsimd.dma_start(W1s[:], W1_d.ap())
        b1s = const.tile([HID, 1], f32)
        nc.gpsimd.dma_start(b1s[:], b1_d.ap())
        W2s = const.tile([HID, WCOLS], bf16)
        nc.gpsimd.dma_start(W2s[:, 0:1024], W2_d.ap()[:, 0:1024])
        nc.gpsimd.dma_start(W2s[:, 1024:], W2_d.ap()[:, 1024:])
        b2s = const.tile([MUL_SRC, NLV], bf16)
        nc.gpsimd.dma_start(b2s[:], b2_d.ap())
        shcs = const.tile([T, 8], f32)
        nc.gpsimd.dma_start(shcs[:], shc_d.ap())

        es_all = const.tile([ESD, EP], bf16)
        nc.sync.dma_start(es_all[:, 0:BLK], esT_d.ap()[:, 0:BLK])
        x_all = const.tile([T, NT, MUL_SRC], bf16)
        nc.sync.dma_start(x_all[:, 0:4, :], x_d.ap()[:, 0:4, :])
        ev_all = const.tile([T, NT, 3], f32)
        nc.sync.dma_start(ev_all[:], ev_d.ap())
        isl_all = const.tile([T, NT], f32)
        nc.sync.dma_start(isl_all[:], isl_d.ap())
        oh_all = const.tile([T, NT, T], bf16)
        nc.sync.dma_start(oh_all[:, 0:4, :], oh_d.ap()[:, 0:4, :])
        xT_all = const.tile([MUL_SRC, EP], bf16)
        nc.sync.dma_start(xT_all[:, 0 : 4 * T], xT_d.ap()[:, 0 : 4 * T])
        nc.sync.dma_start(es_all[:, BLK:], esT_d.ap()[:, BLK:])
        nc.sync.dma_start(x_all[:, 4:, :], x_d.ap()[:, 4:, :])
        nc.sync.dma_start(oh_all[:, 4:, :], oh_d.ap()[:, 4:, :])
        nc.sync.dma_start(xT_all[:, 4 * T :], xT_d.ap()[:, 4 * T :])
        negone = const.tile([T, 1], f32)
        nc.vector.memset(negone[:], -1.0)

        ob_all = const.tile([T, NT, 144], f32)

        # ---- SH prologue: all edges at once, [128, NT, k] layouts ----
        sq_all = shp.tile([T, NT, 3], f32)
        nc.vector.tensor_tensor(sq_all[:], ev_all[:], ev_all[:], op=OP.mult)
        r2_all = shp.tile([T, NT], f32)
        nc.vector.tensor_reduce(r2_all[:], sq_all[:], axis=AX.X, op=OP.add)
        rn_all = shp.tile([T, NT], f32)
        nc.scalar.activation(rn_all[:], r2_all[:], AF.Sqrt)
        inv_all = shp.tile([T, NT], f32)
        nc.vector.reciprocal(inv_all[:], rn_all[:])
        inv2_all = shp.tile([T, NT], f32)
        nc.vector.tensor_tensor(inv2_all[:], inv_all[:], inv_all[:], op=OP.mult)

        def bc(ap_, shape):
            return ap_.to_broadcast(shape)

        sh_all = shp.tile([T, NT, 9], f32)
        i1 = inv_all[:].rearrange("p (t o) -> p t o", o=1)
        i2 = inv2_all[:].rearrange("p (t o) -> p t o", o=1)
        nc.vector.tensor_tensor(
            sh_all[:, :, 1:4], ev_all[:], bc(i1, [T, NT, 3]), op=OP.mult
        )
        pq_all = shp.tile([T, NT, 2], f32)
        nc.vector.tensor_tensor(
            pq_all[:], ev_all[:, :, 0:2], ev_all[:, :, 1:3], op=OP.mult
        )
        nc.vector.tensor_tensor(
            sh_all[:, :, 4:6], pq_all[:], bc(i2, [T, NT, 2]), op=OP.mult
        )
        t6_all = shp.tile([T, NT], f32)
        nc.vector.tensor_tensor(
            t6_all[:].rearrange("p (t o) -> p t o", o=1),
            sq_all[:, :, 2:3],
            i2,
            op=OP.mult,
        )
        nc.scalar.activation(
            sh_all[:, :, 6], t6_all[:], AF.Identity, bias=negone[:, 0:1], scale=3.0
        )
        xz_all = shp.tile([T, NT, 1], f32)
        nc.vector.tensor_tensor(
            xz_all[:], ev_all[:, :, 0:1], ev_all[:, :, 2:3], op=OP.mult
        )
        nc.vector.tensor_tensor(sh_all[:, :, 7:8], xz_all[:], i2, op=OP.mult)
        d2_all = shp.tile([T, NT, 1], f32)
        nc.vector.tensor_tensor(
            d2_all[:], sq_all[:, :, 0:1], sq_all[:, :, 1:2], op=OP.subtract
        )
        nc.vector.tensor_tensor(sh_all[:, :, 8:9], d2_all[:], i2, op=OP.mult)
        shc3 = shcs[:].rearrange("p (o c) -> p o c", o=1)
        nc.vector.tensor_tensor(
            sh_all[:, :, 1:9], sh_all[:, :, 1:9], bc(shc3, [T, NT, 8]), op=OP.mult
        )

        # ---- main pipeline: head(t) emits matmuls/cast/mul/half; the tail
        # (reduce/msg/scatter/evict) is deferred one tile so each engine's
        # in-order stream never waits on the producing engine's latest op.
        xb_by_block = {}

        def tail(st):
            t, halves, tt = st
            xb = xb_by_block[t // 4]
            featc = msgp.tile([T, NLV], f32, tag="featc", name=f"featc{t}")
            for l in range(3):
                nc.vector.tensor_reduce(
                    featc[:, l * MUL_DST : (l + 1) * MUL_DST],
                    halves[l][:],
                    axis=AX.X,
                    op=OP.add,
                )
            # + x @ b2r
            nc.vector.tensor_tensor(
                featc[:], featc[:], xb[:, tt * NLV : (tt + 1) * NLV], op=OP.add
            )
            msg = msgp.tile([T, 144], bf16, tag="msg", name=f"msg{t}")
            nc.vector.tensor_copy(msg[:, 0:16], featc[:, 0:16])
            nc.gpsimd.tensor_tensor(
                msg[:, 16:64].rearrange("p (v m) -> p v m", m=3),
                featc[:, 16:32]
                .rearrange("p (v o) -> p v o", o=1)
                .to_broadcast([T, 16, 3]),
                sh_all[:, t, 1:4]
                .rearrange("p (o m) -> p o m", o=1)
                .to_broadcast([T, 16, 3]),
                op=OP.mult,
            )
            nc.gpsimd.tensor_tensor(
                msg[:, 64:144].rearrange("p (v m) -> p v m", m=5),
                featc[:, 32:48]
                .rearrange("p (v o) -> p v o", o=1)
                .to_broadcast([T, 16, 5]),
                sh_all[:, t, 4:9]
                .rearrange("p (o m) -> p o m", o=1)
                .to_broadcast([T, 16, 5]),
                op=OP.mult,
            )
            ps = psp.tile([T, 144], f32, tag="ps", name=f"ps{t}")
            nc.tensor.matmul(ps[:], oh_all[:, t, :], msg[:], start=True, stop=True)
            nc.scalar.activation(
                ob_all[:, t, :], ps[:], AF.Copy, scale=isl_all[:, t : t + 1]
            )

        pending = []
        for t in range(NT):
            b, tt = divmod(t, 4)
            nbt = min(4, NT - b * 4)  # tiles in this block
            bw = nbt * T  # block width in edges
            if tt == 0:
                h1 = ph1.tile([HID, bw], f32, tag="h1", name=f"h1_{b}")
                nc.tensor.matmul(
                    h1[:],
                    W1s[:],
                    es_all[:, b * BLK : b * BLK + bw],
                    start=True,
                    stop=True,
                )
                h2 = h2pool.tile([HID, bw], bf16, tag="h2", name=f"h2_{b}")
                nc.scalar.activation(h2[:], h1[:], AF.Silu, bias=b1s[:, 0:1])
            if tt == min(1, nbt - 1):
                xbp = pxb.tile([T, 4 * NLV], f32, tag="xbp", name=f"xbp{b}")
                for q in range(nbt):
                    nc.tensor.matmul(
                        xbp[:, q * NLV : (q + 1) * NLV],
                        xT_all[:, (b * 4 + q) * T : (b * 4 + q + 1) * T],
                        b2s[:],
                        start=True,
                        stop=True,
                    )
                xb = msgp.tile([T, 4 * NLV], f32, tag="xb", name=f"xb{b}")
                nc.vector.tensor_copy(xb[:], xbp[:])
                xb_by_block[b] = xb

            lhs = h2[:, tt * T : (tt + 1) * T]
            xv = x_all[:, t : t + 1, :].to_broadcast([T, MUL_DST, MUL_SRC])
            prods = []
            halves = []
            for l in range(3):
                pw = pwp.tile([T, 1024], f32, tag="pw", name=f"pw{t}_{l}")
                for h in range(2):
                    nc.tensor.matmul(
                        pw[:, h * 512 : (h + 1) * 512],
                        lhs,
                        W2s[:, l * 1024 + h * 512 : l * 1024 + (h + 1) * 512],
                        start=True,
                        stop=True,
                    )
                wb = prodp.tile([T, 1024], bf16, tag="wb", name=f"wb{t}_{l}")
                nc.scalar.activation(wb[:], pw[:], AF.Copy)
                prod = prodp.tile([T, 1024], bf16, tag="prod", name=f"prod{t}_{l}")
                wb3 = wb.rearrange("p (v u) -> p v u", u=MUL_SRC)
                pr3 = prod.rearrange("p (v u) -> p v u", u=MUL_SRC)
                nc.vector.tensor_tensor(pr3, wb3, xv, op=OP.mult)
                prods.append(pr3)
            for l in range(3):
                half = prodp.tile(
                    [T, MUL_DST, 32], f32, tag="half", bufs=12, name=f"half{t}_{l}"
                )
                nc.gpsimd.tensor_tensor(
                    half[:], prods[l][:, :, 0:32], prods[l][:, :, 32:64], op=OP.add
                )
                halves.append(half)

            pending.append((t, halves, tt))
            if len(pending) > 2:
                tail(pending.pop(0))
        for st in pending:
            tail(st)

        # output: chunked DMAs, small final chunk so the tail drains fast
        bounds = [0, 12, 24, 36, 44, 48, NT]
        for c in range(len(bounds) - 1):
            nc.sync.dma_start(
                out_d.ap()[:, bounds[c] : bounds[c + 1], :],
                ob_all[:, bounds[c] : bounds[c + 1], :],
            )

    nc.compile()
    return nc


def _get_program():
    global _PROG
    if _PROG is None:
        _PROG = _build_program()
    return _PROG


def _prep_core(c, h_src, edge_vec, edge_scalars, src_idx, dst_idx, inv_cnt):
    """Shard + sort + gather + one-hot build for one core (partition-major)."""
    import ml_dtypes

    bf = ml_dtypes.bfloat16
    lo, hi = c * EC, (c + 1) * EC
    d = dst_idx[lo:hi]
    order = np.argsort(d, kind="stable")
    d_s = d[order]
    s_s = src_idx[lo:hi][order]

    esT = np.zeros((ESD, EP), np.float32)
    esT[:, :EC] = edge_scalars[lo:hi][order].T
    x = np.zeros((EP, MUL_SRC), np.float32)
    x[:EC] = h_src[s_s]
    ev = np.zeros((EP, 3), np.float32)
    ev[:EC] = edge_vec[lo:hi][order]
    ev[EC:, 0] = 1.0

    d_pad = np.full(EP, N_DST, np.int64)
    d_pad[:EC] = d_s

    oh = np.zeros((EP, T), np.float32)
    isl = np.ones((EP,), np.float32)
    labels = np.full(NT * T, N_DST, np.int64)
    dt2 = d_pad.reshape(NT, T)
    for t in range(NT):
        uniq, inv = np.unique(dt2[t], return_inverse=True)
        oh[t * T : (t + 1) * T, :][np.arange(T), inv] = 1.0
        labels[t * T : t * T + len(uniq)] = uniq
        real = uniq[uniq < N_DST]
        isl[t * T : t * T + len(real)] = inv_cnt[real]

    # partition-major device layouts: [p, t, ...] = row t*T + p
    def pmaj(a):
        return np.ascontiguousarray(a.reshape(NT, T, -1).transpose(1, 0, 2))

    return (
        {
            "esT": esT.astype(bf),
            "xg": pmaj(x).astype(bf),
            "xgT": np.ascontiguousarray(x.T).astype(bf),
            "ev": pmaj(ev),
            "oh": pmaj(oh).astype(bf),
            "isl": np.ascontiguousarray(isl.reshape(NT, T).T),
        },
        labels,
    )


def kernel(**inputs):
    import ml_dtypes

    from concourse import bass_utils

    bf = ml_dtypes.bfloat16

    h_src = np.asarray(inputs["h_src"], np.float32)
    edge_vec = np.asarray(inputs["edge_vec"], np.float32)
    edge_scalars = np.asarray(inputs["edge_scalars"], np.float32)
    W1 = np.asarray(inputs["W1"], np.float32)
    b1 = np.asarray(inputs["b1"], np.float32)
    W2 = np.asarray(inputs["W2"], np.float32)
    b2 = np.asarray(inputs["b2"], np.float32)
    src_idx = np.asarray(inputs["src_idx"]).astype(np.int64)
    dst_idx = np.asarray(inputs["dst_idx"]).astype(np.int64)
    n_dst = int(inputs["n_dst"])
    assert n_dst == N_DST

    nc = _get_program()

    cnt = np.bincount(dst_idx, minlength=N_DST)
    inv_cnt = (1.0 / np.maximum(cnt, 1)).astype(np.float32)

    # weights in (l,v,u) column order, pre-scaled by 1/sqrt(64)
    scale = 1.0 / np.sqrt(MUL_SRC)
    W2p = (
        W2.reshape(HID, N_PATHS, MUL_SRC, MUL_DST).transpose(0, 1, 3, 2) * scale
    ).reshape(HID, WCOLS)
    b2r = (b2.reshape(N_PATHS, MUL_SRC, MUL_DST).transpose(1, 0, 2) * scale).reshape(
        MUL_SRC, NLV
    )

    shc = np.broadcast_to(
        np.array(
            [SQ3, SQ3, SQ3, SQ15, SQ15, 0.5 * SQ5, SQ15, 0.5 * SQ15], np.float32
        ),
        (T, 8),
    ).copy()

    shared = {
        "W1": np.ascontiguousarray(W1).astype(bf),
        "b1": b1.reshape(HID, 1).astype(np.float32),
        "W2p": W2p.astype(bf),
        "b2r": b2r.astype(bf),
        "shc": shc,
    }

    in_maps = []
    labels_all = []
    for c in range(N_CORES):
        m, labels = _prep_core(
            c, h_src, edge_vec, edge_scalars, src_idx, dst_idx, inv_cnt
        )
        m.update(shared)
        in_maps.append(m)
        labels_all.append(labels)

    import time

    t0 = time.perf_counter()
    res = bass_utils.run_bass_kernel_spmd(nc, in_maps, core_ids=list(range(N_CORES)))
    t1 = time.perf_counter()
    kernel.last_device_wall_s = t1 - t0

    # outp is [T, NT, 144] partition-major; row (t, p) lives at [p, t, :]
    rows = np.concatenate(
        [
            res.results[c]["outp"].transpose(1, 0, 2).reshape(NT * T, 144)
            for c in range(N_CORES)
        ],
        axis=0,
    )
    labels = np.concatenate(labels_all)

    order = np.argsort(labels, kind="stable")
    lab_s = labels[order]
    rows_s = rows[order]
    starts = np.concatenate(([0], np.flatnonzero(np.diff(lab_s)) + 1))
    sums = np.add.reduceat(rows_s, starts, axis=0)
    out = np.zeros((N_DST + 1, 144), np.float32)
    out[lab_s[starts]] = sums
    return out[:N_DST]


# revision 34
# speedup vs baseline: 1.0565x; 1.0549x over previous
"""Trainium2 Bass kernel for EquivariantTPConv (gnn_message_passing).

Computation per edge e:
  sh  = SH_l012(edge_vec[e])                                  # [9]
  w   = (silu(edge_scalars[e] @ W1 + b1) @ W2 + b2)           # [3*64*16]
  x   = h_src[src_idx[e]]                                     # [64]
  feat[l,v] = sum_u x[u] * w[l,u,v] / 8                       # [3,16]
  msg = concat_l (feat[:,l,:,None] * sh_l[None,:])            # [144]
  out[d] = mean over {e: dst_idx[e]==d} msg[e]                # [n_dst,144]

Strategy (8 NeuronCores, edge/data parallel):
  - Host shards edges into 8 contiguous chunks, sorts each shard by dst,
    gathers h_src rows per shard (sharding/data movement only), and builds
    per-tile one-hot "slot" matrices (slot = distinct dst within a 128-edge
    tile) plus 1/count scales. All per-core arrays are stored partition-major
    so each lands on the device in one big DMA.
  - Device (per core): SH for all edges upfront in wide [128, NT, k] ops
    (Vector+Scalar engines); MLP on TensorE in bf16 (h1^T = W1^T @ es^T so
    the silu output is directly the lhsT of the big matmul); w = h2 @ W2perm
    on TensorE (fp32 PSUM); ScalarE casts w to bf16 SBUF; VectorE multiplies
    by the gathered x (2x bf16 mode); GpSimd adds the two u-halves; VectorE
    reduces the remaining 32; one-hot matmul on TensorE does the within-tile
    segment-sum; 1/count is applied during the PSUM->SBUF eviction.
  - Host unshard: segment-sum of the (sorted-label) slot rows across tiles
    and cores via np.add.reduceat == the cross-core all-reduce step.

W2 is column-permuted (l,u,v)->(l,v,u) and pre-scaled by 1/sqrt(64) on the
host so u is contiguous for the contraction; b2 likewise reshaped to [64,48]
and handled exactly via one extra tiny matmul x @ b2r per tile.
"""

import sys

for _p in ("/opt/trn_rl_repo", "/root/.axon_site/_ro/trn_rl_repo"):
    if _p not in sys.path:
        sys.path.append(_p)

import numpy as np

MUL_SRC = 64
MUL_DST = 16
N_PATHS = 3
SQ3 = 3.0 ** 0.5
SQ5 = 5.0 ** 0.5
SQ15 = 15.0 ** 0.5

N_CORES = 8
E_TOT = 50000
N_SRC = 10000
N_DST = 10000
ESD = 32
HID = 128
WCOLS = N_PATHS * MUL_DST * MUL_SRC  # 3072 (perm layout (l,v,u))
NLV = N_PATHS * MUL_DST  # 48

T = 128  # edges per tile
BLK = 512  # edges per full MM1 block (last block is a 128-edge tail)
EC = E_TOT // N_CORES  # 6250 edges per core
NT = (EC + T - 1) // T  # 49 tiles
EP = NT * T  # 6272 padded edges per core
NB = (EP + BLK - 1) // BLK  # 13 blocks, last one covers a single tile

_PROG = None  # cached compiled program


def _build_program():
    from contextlib import ExitStack

    import concourse.tile as tile
    from concourse import bacc, mybir

    f32 = mybir.dt.float32
    bf16 = mybir.dt.bfloat16
    AF = mybir.ActivationFunctionType
    OP = mybir.AluOpType
    AX = mybir.AxisListType

    nc = bacc.Bacc(
        "TRN2",
        target_bir_lowering=False,
        debug=False,
        enable_asserts=False,
        num_devices=N_CORES,
    )

    # DRAM inputs; all big per-core tensors are partition-major on the host.
    esT_d = nc.dram_tensor("esT", [ESD, EP], bf16, kind="ExternalInput")
    x_d = nc.dram_tensor("xg", [T, NT, MUL_SRC], bf16, kind="ExternalInput")
    xT_d = nc.dram_tensor("xgT", [MUL_SRC, EP], bf16, kind="ExternalInput")
    ev_d = nc.dram_tensor("ev", [T, NT, 3], f32, kind="ExternalInput")
    oh_d = nc.dram_tensor("oh", [T, NT, T], bf16, kind="ExternalInput")
    isl_d = nc.dram_tensor("isl", [T, NT], f32, kind="ExternalInput")
    W1_d = nc.dram_tensor("W1", [ESD, HID], bf16, kind="ExternalInput")
    b1_d = nc.dram_tensor("b1", [HID, 1], f32, kind="ExternalInput")
    W2_d = nc.dram_tensor("W2p", [HID, WCOLS], bf16, kind="ExternalInput")
    b2_d = nc.dram_tensor("b2r", [MUL_SRC, NLV], bf16, kind="ExternalInput")
    shc_d = nc.dram_tensor("shc", [T, 8], f32, kind="ExternalInput")
    out_d = nc.dram_tensor("outp", [T, NT, 144], f32, kind="ExternalOutput")

    with ExitStack() as ctx:
        tc = ctx.enter_context(tile.TileContext(nc))

        const = ctx.enter_context(tc.tile_pool(name="const", bufs=1))
        shp = ctx.enter_context(tc.tile_pool(name="shp", bufs=1))
        h2pool = ctx.enter_context(tc.tile_pool(name="h2pool", bufs=4))
        prodp = ctx.enter_context(tc.tile_pool(name="prodp", bufs=8))
        msgp = ctx.enter_context(tc.tile_pool(name="msgp", bufs=6))
        ph1 = ctx.enter_context(tc.tile_pool(name="ph1", bufs=2, space="PSUM"))
        pwp = ctx.enter_context(tc.tile_pool(name="pwp", bufs=2, space="PSUM"))
        psp = ctx.enter_context(tc.tile_pool(name="psp", bufs=1, space="PSUM"))
        pxb = ctx.enter_context(tc.tile_pool(name="pxb", bufs=1, space="PSUM"))

        # ---- resident loads; SP carries the edge streams (it has no compute),
        # GpSimd's queue carries the weights, ordered by first use ----
        W1s = const.tile([ESD, HID], bf16)
        nc.gpsimd.dma_start(W1s[:], W1_d.ap())
        b1s = const.tile([HID, 1], f32)
        nc.gpsimd.dma_start(b1s[:], b1_d.ap())
        W2s = const.tile([HID, WCOLS], bf16)
        nc.gpsimd.dma_start(W2s[:, 0:1024], W2_d.ap()[:, 0:1024])
        nc.gpsimd.dma_start(W2s[:, 1024:], W2_d.ap()[:, 1024:])
        b2s = const.tile([MUL_SRC, NLV], bf16)
        nc.gpsimd.dma_start(b2s[:], b2_d.ap())
        shcs = const.tile([T, 8], f32)
        nc.gpsimd.dma_start(shcs[:], shc_d.ap())

        es_all = const.tile([ESD, EP], bf16)
        nc.sync.dma_start(es_all[:, 0:BLK], esT_d.ap()[:, 0:BLK])
        x_all = const.tile([T, NT, MUL_SRC], bf16)
        nc.sync.dma_start(x_all[:, 0:4, :], x_d.ap()[:, 0:4, :])
        ev_all = const.tile([T, NT, 3], f32)
        nc.sync.dma_start(ev_all[:], ev_d.ap())
        isl_all = const.tile([T, NT], f32)
        nc.sync.dma_start(isl_all[:], isl_d.ap())
        oh_all = const.tile([T, NT, T], bf16)
        nc.sync.dma_start(oh_all[:, 0:4, :], oh_d.ap()[:, 0:4, :])
        xT_all = const.tile([MUL_SRC, EP], bf16)
        nc.sync.dma_start(xT_all[:, 0 : 4 * T], xT_d.ap()[:, 0 : 4 * T])
        nc.sync.dma_start(es_all[:, BLK:], esT_d.ap()[:, BLK:])
        nc.sync.dma_start(x_all[:, 4:, :], x_d.ap()[:, 4:, :])
        nc.sync.dma_start(oh_all[:, 4:, :], oh_d.ap()[:, 4:, :])
        nc.sync.dma_start(xT_all[:, 4 * T :], xT_d.ap()[:, 4 * T :])
        negone = const.tile([T, 1], f32)
        nc.vector.memset(negone[:], -1.0)

        ob_all = const.tile([T, NT, 144], f32)

        # ---- SH prologue: all edges at once, [128, NT, k] layouts ----
        sq_all = shp.tile([T, NT, 3], f32)
        nc.vector.tensor_tensor(sq_all[:], ev_all[:], ev_all[:], op=OP.mult)
        r2_all = shp.tile([T, NT], f32)
        nc.vector.tensor_reduce(r2_all[:], sq_all[:], axis=AX.X, op=OP.add)
        rn_all = shp.tile([T, NT], f32)
        nc.scalar.activation(rn_all[:], r2_all[:], AF.Sqrt)
        def bc(ap_, shape):
            return ap_.to_broadcast(shape)

        sh_all = shp.tile([T, NT, 9], f32)

        def emit_sh_part2():
            inv_all = shp.tile([T, NT], f32)
            nc.vector.reciprocal(inv_all[:], rn_all[:])
            inv2_all = shp.tile([T, NT], f32)
            nc.vector.tensor_tensor(inv2_all[:], inv_all[:], inv_all[:], op=OP.mult)
            i1 = inv_all[:].rearrange("p (t o) -> p t o", o=1)
            i2 = inv2_all[:].rearrange("p (t o) -> p t o", o=1)
            nc.vector.tensor_tensor(
                sh_all[:, :, 1:4], ev_all[:], bc(i1, [T, NT, 3]), op=OP.mult
            )
            pq_all = shp.tile([T, NT, 2], f32)
            nc.vector.tensor_tensor(
                pq_all[:], ev_all[:, :, 0:2], ev_all[:, :, 1:3], op=OP.mult
            )
            nc.vector.tensor_tensor(
                sh_all[:, :, 4:6], pq_all[:], bc(i2, [T, NT, 2]), op=OP.mult
            )
            t6_all = shp.tile([T, NT], f32)
            nc.vector.tensor_tensor(
                t6_all[:].rearrange("p (t o) -> p t o", o=1),
                sq_all[:, :, 2:3],
                i2,
                op=OP.mult,
            )
            nc.scalar.activation(
                sh_all[:, :, 6], t6_all[:], AF.Identity, bias=negone[:, 0:1], scale=3.0
            )
            xz_all = shp.tile([T, NT, 1], f32)
            nc.vector.tensor_tensor(
                xz_all[:], ev_all[:, :, 0:1], ev_all[:, :, 2:3], op=OP.mult
            )
            nc.vector.tensor_tensor(sh_all[:, :, 7:8], xz_all[:], i2, op=OP.mult)
            d2_all = shp.tile([T, NT, 1], f32)
            nc.vector.tensor_tensor(
                d2_all[:], sq_all[:, :, 0:1], sq_all[:, :, 1:2], op=OP.subtract
            )
            nc.vector.tensor_tensor(sh_all[:, :, 8:9], d2_all[:], i2, op=OP.mult)
            shc3 = shcs[:].rearrange("p (o c) -> p o c", o=1)
            nc.vector.tensor_tensor(
                sh_all[:, :, 1:9], sh_all[:, :, 1:9], bc(shc3, [T, NT, 8]), op=OP.mult
            )

        # ---- main pipeline: head(t) emits matmuls/cast/mul/half; the tail
        # (reduce/msg/scatter/evict) is deferred one tile so each engine's
        # in-order stream never waits on the producing engine's latest op.
        xb_by_block = {}

        def tail(st):
            t, halves, tt = st
            xb = xb_by_block[t // 4]
            featc = msgp.tile([T, NLV], f32, tag="featc", name=f"featc{t}")
            for l in range(3):
                nc.vector.tensor_reduce(
                    featc[:, l * MUL_DST : (l + 1) * MUL_DST],
                    halves[l][:],
                    axis=AX.X,
                    op=OP.add,
                )
            # + x @ b2r
            nc.vector.tensor_tensor(
                featc[:], featc[:], xb[:, tt * NLV : (tt + 1) * NLV], op=OP.add
            )
            msg = msgp.tile([T, 144], bf16, tag="msg", name=f"msg{t}")
            nc.vector.tensor_copy(msg[:, 0:16], featc[:, 0:16])
            nc.gpsimd.tensor_tensor(
                msg[:, 16:64].rearrange("p (v m) -> p v m", m=3),
                featc[:, 16:32]
                .rearrange("p (v o) -> p v o", o=1)
                .to_broadcast([T, 16, 3]),
                sh_all[:, t, 1:4]
                .rearrange("p (o m) -> p o m", o=1)
                .to_broadcast([T, 16, 3]),
                op=OP.mult,
            )
            nc.gpsimd.tensor_tensor(
                msg[:, 64:144].rearrange("p (v m) -> p v m", m=5),
                featc[:, 32:48]
                .rearrange("p (v o) -> p v o", o=1)
                .to_broadcast([T, 16, 5]),
                sh_all[:, t, 4:9]
                .rearrange("p (o m) -> p o m", o=1)
                .to_broadcast([T, 16, 5]),
                op=OP.mult,
            )
            ps = psp.tile([T, 144], f32, tag="ps", name=f"ps{t}")
            nc.tensor.matmul(ps[:], oh_all[:, t, :], msg[:], start=True, stop=True)
            nc.scalar.activation(
                ob_all[:, t, :], ps[:], AF.Copy, scale=isl_all[:, t : t + 1]
            )

        pending = []
        for t in range(NT):
            b, tt = divmod(t, 4)
            nbt = min(4, NT - b * 4)  # tiles in this block
            bw = nbt * T  # block width in edges
            if tt == 0:
                h1 = ph1.tile([HID, bw], f32, tag="h1", name=f"h1_{b}")
                nc.tensor.matmul(
                    h1[:],
                    W1s[:],
                    es_all[:, b * BLK : b * BLK + bw],
                    start=True,
                    stop=True,
                )
                h2 = h2pool.tile([HID, bw], bf16, tag="h2", name=f"h2_{b}")
                nc.scalar.activation(h2[:], h1[:], AF.Silu, bias=b1s[:, 0:1])
            if tt == min(1, nbt - 1):
                xbp = pxb.tile([T, 4 * NLV], f32, tag="xbp", name=f"xbp{b}")
                for q in range(nbt):
                    nc.tensor.matmul(
                        xbp[:, q * NLV : (q + 1) * NLV],
                        xT_all[:, (b * 4 + q) * T : (b * 4 + q + 1) * T],
                        b2s[:],
                        start=True,
                        stop=True,
                    )
                xb = msgp.tile([T, 4 * NLV], f32, tag="xb", name=f"xb{b}")
                nc.vector.tensor_copy(xb[:], xbp[:])
                xb_by_block[b] = xb

            lhs = h2[:, tt * T : (tt + 1) * T]
            xv = x_all[:, t : t + 1, :].to_broadcast([T, MUL_DST, MUL_SRC])
            prods = []
            halves = []
            for l in range(3):
                pw = pwp.tile([T, 1024], f32, tag="pw", name=f"pw{t}_{l}")
                for h in range(2):
                    nc.tensor.matmul(
                        pw[:, h * 512 : (h + 1) * 512],
                        lhs,
                        W2s[:, l * 1024 + h * 512 : l * 1024 + (h + 1) * 512],
                        start=True,
                        stop=True,
                    )
                wb = prodp.tile([T, 1024], bf16, tag="wb", name=f"wb{t}_{l}")
                nc.scalar.activation(wb[:], pw[:], AF.Copy)
                prod = prodp.tile([T, 1024], bf16, tag="prod", name=f"prod{t}_{l}")
                wb3 = wb.rearrange("p (v u) -> p v u", u=MUL_SRC)
                pr3 = prod.rearrange("p (v u) -> p v u", u=MUL_SRC)
                nc.vector.tensor_tensor(pr3, wb3, xv, op=OP.mult)
                prods.append(pr3)
            for l in range(3):
                half = prodp.tile(
                    [T, MUL_DST, 32], f32, tag="half", bufs=12, name=f"half{t}_{l}"
                )
                nc.gpsimd.tensor_tensor(
                    half[:], prods[l][:, :, 0:32], prods[l][:, :, 32:64], op=OP.add
                )
                halves.append(half)

            if t == 1:
                emit_sh_part2()
            pending.append((t, halves, tt))
            if len(pending) > 2:
                tail(pending.pop(0))
        for st in pending:
            tail(st)

        # output: chunked DMAs, small final chunk so the tail drains fast
        bounds = [0, 12, 24, 36, 44, 48, NT]
        for c in range(len(bounds) - 1):
            nc.sync.dma_start(
                out_d.ap()[:, bounds[c] : bounds[c + 1], :],
                ob_all[:, bounds[c] : bounds[c + 1], :],
            )

    nc.compile()
    return nc


def _get_program():
    global _PROG
    if _PROG is None:
        _PROG = _build_program()
    return _PROG


def _prep_core(c, h_src, edge_vec, edge_scalars, src_idx, dst_idx, inv_cnt):
    """Shard + sort + gather + one-hot build for one core (partition-major)."""
    import ml_dtypes

    bf = ml_dtypes.bfloat16
    lo, hi = c * EC, (c + 1) * EC
    d = dst_idx[lo:hi]
    order = np.argsort(d, kind="stable")
    d_s = d[order]
    s_s = src_idx[lo:hi][order]

    esT = np.zeros((ESD, EP), np.float32)
    esT[:, :EC] = edge_scalars[lo:hi][order].T
    x = np.zeros((EP, MUL_SRC), np.float32)
    x[:EC] = h_src[s_s]
    ev = np.zeros((EP, 3), np.float32)
    ev[:EC] = edge_vec[lo:hi][order]
    ev[EC:, 0] = 1.0

    d_pad = np.full(EP, N_DST, np.int64)
    d_pad[:EC] = d_s

    oh = np.zeros((EP, T), np.float32)
    isl = np.ones((EP,), np.float32)
    labels = np.full(NT * T, N_DST, np.int64)
    dt2 = d_pad.reshape(NT, T)
    for t in range(NT):
        uniq, inv = np.unique(dt2[t], return_inverse=True)
        oh[t * T : (t + 1) * T, :][np.arange(T), inv] = 1.0
        labels[t * T : t * T + len(uniq)] = uniq
        real = uniq[uniq < N_DST]
        isl[t * T : t * T + len(real)] = inv_cnt[real]

    # partition-major device layouts: [p, t, ...] = row t*T + p
    def pmaj(a):
        return np.ascontiguousarray(a.reshape(NT, T, -1).transpose(1, 0, 2))

    return (
        {
            "esT": esT.astype(bf),
            "xg": pmaj(x).astype(bf),
            "xgT": np.ascontiguousarray(x.T).astype(bf),
            "ev": pmaj(ev),
            "oh": pmaj(oh).astype(bf),
            "isl": np.ascontiguousarray(isl.reshape(NT, T).T),
        },
        labels,
    )


def kernel(**inputs):
    import ml_dtypes

    from concourse import bass_utils

    bf = ml_dtypes.bfloat16

    h_src = np.asarray(inputs["h_src"], np.float32)
    edge_vec = np.asarray(inputs["edge_vec"], np.float32)
    edge_scalars = np.asarray(inputs["edge_scalars"], np.float32)
    W1 = np.asarray(inputs["W1"], np.float32)
    b1 = np.asarray(inputs["b1"], np.float32)
    W2 = np.asarray(inputs["W2"], np.float32)
    b2 = np.asarray(inputs["b2"], np.float32)
    src_idx = np.asarray(inputs["src_idx"]).astype(np.int64)
    dst_idx = np.asarray(inputs["dst_idx"]).astype(np.int64)
    n_dst = int(inputs["n_dst"])
    assert n_dst == N_DST

    nc = _get_program()

    cnt = np.bincount(dst_idx, minlength=N_DST)
    inv_cnt = (1.0 / np.maximum(cnt, 1)).astype(np.float32)

    # weights in (l,v,u) column order, pre-scaled by 1/sqrt(64)
    scale = 1.0 / np.sqrt(MUL_SRC)
    W2p = (
        W2.reshape(HID, N_PATHS, MUL_SRC, MUL_DST).transpose(0, 1, 3, 2) * scale
    ).reshape(HID, WCOLS)
    b2r = (b2.reshape(N_PATHS, MUL_SRC, MUL_DST).transpose(1, 0, 2) * scale).reshape(
        MUL_SRC, NLV
    )

    shc = np.broadcast_to(
        np.array(
            [SQ3, SQ3, SQ3, SQ15, SQ15, 0.5 * SQ5, SQ15, 0.5 * SQ15], np.float32
        ),
        (T, 8),
    ).copy()

    shared = {
        "W1": np.ascontiguousarray(W1).astype(bf),
        "b1": b1.reshape(HID, 1).astype(np.float32),
        "W2p": W2p.astype(bf),
        "b2r": b2r.astype(bf),
        "shc": shc,
    }

    in_maps = []
    labels_all = []
    for c in range(N_CORES):
        m, labels = _prep_core(
            c, h_src, edge_vec, edge_scalars, src_idx, dst_idx, inv_cnt
        )
        m.update(shared)
        in_maps.append(m)
        labels_all.append(labels)

    import time

    t0 = time.perf_counter()
    res = bass_utils.run_bass_kernel_spmd(nc, in_maps, core_ids=list(range(N_CORES)))
    t1 = time.perf_counter()
    kernel.last_device_wall_s = t1 - t0

    # outp is [T, NT, 144] partition-major; row (t, p) lives at [p, t, :]
    rows = np.concatenate(
        [
            res.results[c]["outp"].transpose(1, 0, 2).reshape(NT * T, 144)
            for c in range(N_CORES)
        ],
        axis=0,
    )
    labels = np.concatenate(labels_all)

    order = np.argsort(labels, kind="stable")
    lab_s = labels[order]
    rows_s = rows[order]
    starts = np.concatenate(([0], np.flatnonzero(np.diff(lab_s)) + 1))
    sums = np.add.reduceat(rows_s, starts, axis=0)
    out = np.zeros((N_DST + 1, 144), np.float32)
    out[lab_s[starts]] = sums
    return out[:N_DST]
